# revision 36
# baseline (speedup 1.0000x reference)
import os
import sys
import numpy as np

# GaussianUpsampler on 8 NeuronCores (Bass/Tile).
#   out[b] = rownorm(W[b]) @ feats[b],  W[b][o,t] = N(o; c[t], r[t]) + 1e-6
#   B=32, T=512, D=384, outlen ~ 2360.
# Sharding: data-parallel over batch, 4 batches per core, no collectives.
# Per core/batch: W^T band tiles (token-major = matmul lhsT layout) are
# computed on-chip (DVE z/z^2 + ACT exp) in bf16; PE contracts them against
# feats chunks augmented with a ones column (row sums for normalization) and
# a floor row (the +1e-6 * sum-over-all-tokens term, exact); DVE normalizes
# with a per-partition reciprocal. W is banded: only token chunks whose
# Gaussian centers are within +-MARGIN frames of an output tile take part
# (union across cores so one SPMD program serves all 8).

for _p in ("/opt/trn_rl_repo", "/root/.axon_site/_ro/trn_rl_repo"):
    if os.path.isdir(_p) and _p not in sys.path:
        sys.path.insert(0, _p)
        break

R2PI = float(np.sqrt(2.0 * np.pi))
B, T, D = 32, 512, 384
DA = D + 1            # feats + ones column
P = 128
N_CORES = 8
BPC = B // N_CORES    # local batches per core
NT = (127, 127, 127, 127, 4)   # tokens per K-chunk (one row left for floor)
NCH = len(NT)
TOK0 = (0, 127, 254, 381, 508)
MARGIN = 36.0

_runners = {}


def _upsample_np(feats, rng, durations, outlen):
    d = durations.astype(np.float32)
    c = d / 2.0 + np.cumsum(d, axis=-1)
    r = rng.astype(np.float32) + 1e-6
    t = np.arange(outlen, dtype=np.float32)
    z = (t[None, :, None] - c[:, None, :]) / r[:, None, :]
    w = np.exp(-0.5 * z * z) / (r[:, None, :] * R2PI) + 1e-6
    w /= w.sum(axis=2, keepdims=True)
    return np.matmul(w, feats.astype(np.float32))


def _bands(c, OT):
    """Per (slot, chunk) o-tile ranges, unioned across cores (SPMD)."""
    jlo = np.zeros((BPC, NCH), np.int64)
    jhi = np.zeros((BPC, NCH), np.int64)
    for s in range(BPC):
        for k in range(NCH):
            lo, hi = 10 ** 9, -1
            for m in range(N_CORES):
                b = m * BPC + s
                cmin = float(c[b, TOK0[k]])
                cmax = float(c[b, TOK0[k] + NT[k] - 1])
                lo = min(lo, int(np.floor((cmin - MARGIN) / P)))
                hi = max(hi, int(np.floor((cmax + MARGIN) / P)))
            jlo[s, k] = max(0, lo)
            jhi[s, k] = min(OT - 1, hi)
        # the last (tiny) chunk also carries the floor for tail frames past
        # every batch's final center
        jhi[s, NCH - 1] = OT - 1
    L = [
        [
            [k for k in range(NCH) if jlo[s, k] <= j <= jhi[s, k]]
            for j in range(OT)
        ]
        for s in range(BPC)
    ]
    for s in range(BPC):
        for j in range(OT):
            if not L[s][j]:
                raise ValueError("band coverage hole")  # -> numpy fallback
    return jlo, jhi, L


def _prep(feats, rng, durations, outlen):
    import ml_dtypes

    bf16 = ml_dtypes.bfloat16
    OT = -(-outlen // P)

    dn = durations.astype(np.float32)
    c = dn / 2.0 + np.cumsum(dn, axis=1, dtype=np.float32)
    invr = (1.0 / (rng.astype(np.float32) + 1e-6)).astype(np.float32)
    logg = np.log(invr / R2PI).astype(np.float32)
    negc = (-c * invr).astype(np.float32)

    jlo, jhi, L = _bands(c, OT)

    prm = np.zeros((N_CORES, BPC, P, NCH, 3), np.float32)
    # full chunks packed [P, NCH-1, DA] (per-partition rows contiguous =
    # 1 DMA descriptor per partition); the 5-row tail chunk rides separately
    # so 123 padding rows per batch aren't shipped through the tunnel
    fa = np.zeros((N_CORES, BPC, P, NCH - 1, DA), bf16)
    fa4 = np.zeros((N_CORES, BPC, NT[NCH - 1] + 1, DA), bf16)
    feats_bf = feats.astype(bf16)
    flo = (1e-6 * feats.astype(np.float64).sum(axis=1)).astype(np.float32)
    flo_bf = flo.astype(bf16).reshape(N_CORES, BPC, D)
    for k in range(NCH):
        n = NT[k]
        t0 = TOK0[k]
        sl = np.s_[:, t0 : t0 + n]
        prm[:, :, :n, k, 0] = invr[sl].reshape(N_CORES, BPC, n)
        prm[:, :, :n, k, 1] = negc[sl].reshape(N_CORES, BPC, n)
        prm[:, :, :n, k, 2] = logg[sl].reshape(N_CORES, BPC, n)
        if k < NCH - 1:
            fa[:, :, :n, k, :D] = feats_bf[sl].reshape(N_CORES, BPC, n, D)
            fa[:, :, :n, k, D] = bf16(1.0)
            fa[:, :, n, k, :D] = flo_bf
            fa[:, :, n, k, D] = bf16(T * 1e-6)
        else:
            fa4[:, :, :n, :D] = feats_bf[sl].reshape(N_CORES, BPC, n, D)
            fa4[:, :, :n, D] = bf16(1.0)
            fa4[:, :, n, :D] = flo_bf
            fa4[:, :, n, D] = bf16(T * 1e-6)
    key = (int(outlen), jlo.tobytes(), jhi.tobytes())
    return fa, fa4, prm, jlo, jhi, L, OT, key


def _build(outlen, OT, jlo, jhi, L):
    from concourse import bacc, bass, mybir, tile

    f32 = mybir.dt.float32
    bf16 = mybir.dt.bfloat16
    Act = mybir.ActivationFunctionType
    OPAD = OT * P

    NT4 = NT[NCH - 1] + 1
    nc = bacc.Bacc("TRN2", target_bir_lowering=False, debug=False)
    fa_d = nc.dram_tensor(
        "fa", [BPC, P, NCH - 1, DA], bf16, kind="ExternalInput"
    )
    fa4_d = nc.dram_tensor("fa4", [BPC, NT4, DA], bf16, kind="ExternalInput")
    prm_d = nc.dram_tensor("prm", [BPC, P, NCH, 3], f32, kind="ExternalInput")
    # bf16 output halves the device->host transfer (the wall-clock
    # bottleneck over the axon tunnel); host upcasts to f32.
    out_d = nc.dram_tensor("out", [BPC, outlen, D], bf16, kind="ExternalOutput")
    JF = outlen // P  # full o-tiles; tile JF (if any) is partial

    with tile.TileContext(nc) as tc:
        with (
            tc.tile_pool(name="const", bufs=1) as cpool,
            tc.tile_pool(name="wz", bufs=3) as wzpool,
            tc.tile_pool(name="fap", bufs=2) as fapool,
            tc.tile_pool(name="op", bufs=2) as opool,
            tc.tile_pool(name="rc", bufs=8) as rcpool,
            tc.tile_pool(name="ps", bufs=8, space=bass.MemorySpace.PSUM) as pspool,
        ):
            # dependency-free ACT op up front so the ~2.7us table load runs
            # at t=0, overlapped with iota/param DMAs instead of serialized
            # before the first real Square
            dum_t = cpool.tile([1, 2], f32)
            nc.vector.memset(dum_t[:], 0.0)
            nc.scalar.activation(dum_t[:], dum_t[:], Act.Exp)
            # f32 iota is exact up to 2^24; avoids an int32 staging copy
            iota_t = cpool.tile([P, OPAD], f32)
            nc.gpsimd.iota(
                iota_t[:],
                pattern=[[1, OPAD]],
                base=0,
                channel_multiplier=0,
                allow_small_or_imprecise_dtypes=True,
            )
            for s in range(BPC):
                prm_t = fapool.tile([P, NCH, 3], f32, tag="prm")
                nc.sync.dma_start(prm_t[:], prm_d[s])
                fa_t = fapool.tile([P, NCH - 1, DA], bf16, tag="fa")
                nc.sync.dma_start(fa_t[:], fa_d[s])
                fa4_t = fapool.tile([NT4, DA], bf16, tag="fa4")
                nc.sync.dma_start(fa4_t[:], fa4_d[s])
                w_ts = []
                for k in range(NCH):
                    wk = (int(jhi[s, k]) - int(jlo[s, k]) + 1) * P
                    o0 = int(jlo[s, k]) * P
                    # z^2 = (o*invr - c*invr)^2 in one ACT pass (per-partition
                    # scale/bias); Square+Exp share one table set.
                    z_t = wzpool.tile([P, wk], f32, tag=f"z{k}")
                    nc.scalar.activation(
                        z_t[:],
                        iota_t[:, o0 : o0 + wk],
                        Act.Square,
                        bias=prm_t[:, k, 1:2],
                        scale=prm_t[:, k, 0:1],
                    )
                    w_t = wzpool.tile([P, wk], bf16, tag=f"w{k}")
                    # floor row NT[k]: prm rows are zeroed there, so this
                    # same pass writes exp(-0.5*0 + 0) = 1.0 — the ones row
                    # the floor matmul term needs.
                    nc.scalar.activation(
                        w_t[:], z_t[:], Act.Exp, bias=prm_t[:, k, 2:3], scale=-0.5
                    )
                    w_ts.append(w_t)
                obig_t = opool.tile([P, OT, D], bf16, tag="obig")
                for j in range(OT):
                    ks = L[s][j]
                    ps_t = pspool.tile([P, DA], f32)
                    for i, k in enumerate(ks):
                        rows = NT[k] + (1 if i == 0 else 0)
                        col0 = (j - int(jlo[s, k])) * P
                        rhs = (
                            fa_t[0:rows, k, :]
                            if k < NCH - 1
                            else fa4_t[0:rows, :]
                        )
                        nc.tensor.matmul(
                            ps_t[:],
                            w_ts[k][0:rows, col0 : col0 + P],
                            rhs,
                            start=(i == 0),
                            stop=(i == len(ks) - 1),
                        )
                    rec_t = rcpool.tile([P, 1], f32, tag="rec")
                    nc.vector.reciprocal(rec_t[:], ps_t[:, D:DA])
                    # PSUM drain + normalize + bf16 convert in one pass;
                    # alternate DVE/ACT to balance engine load
                    if j % 13 == 12:
                        nc.scalar.activation(
                            obig_t[:, j, :],
                            ps_t[:, 0:D],
                            Act.Copy,
                            scale=rec_t[:],
                        )
                    else:
                        nc.vector.tensor_scalar_mul(
                            obig_t[:, j, :], ps_t[:, 0:D], rec_t[:]
                        )
                # two DMAs per slot instead of one per o-tile: HWDGE fixed
                # overhead (~625ns/instr) dominated the DMA path otherwise
                nc.sync.dma_start(
                    out_d[s, 0 : JF * P, :].rearrange(
                        "(j p) d -> p j d", p=P
                    ),
                    obig_t[:, 0:JF, :],
                )
                if outlen > JF * P:
                    nc.sync.dma_start(
                        out_d[s, JF * P : outlen, :],
                        obig_t[0 : outlen - JF * P, JF, :],
                    )
    nc.compile()
    return nc


def _make_runner(nc):
    """One-time jit of the SPMD bass_exec call; later calls only transfer."""
    import jax
    import jax.numpy as jnp
    from jax.experimental.shard_map import shard_map
    from jax.sharding import Mesh, NamedSharding, PartitionSpec

    from concourse import bass2jax, mybir

    bass2jax.install_neuronx_cc_hook()
    assert nc.dbg_addr is None
    pname = nc.partition_id_tensor.name if nc.partition_id_tensor else None

    in_names, out_names, out_avals = [], [], []
    for alloc in nc.m.functions[0].allocations:
        if not isinstance(alloc, mybir.MemoryLocationSet):
            continue
        name = alloc.memorylocations[0].name
        if alloc.kind == "ExternalInput":
            if name != pname:
                in_names.append(name)
        elif alloc.kind == "ExternalOutput":
            out_names.append(name)
            out_avals.append(
                jax.core.ShapedArray(
                    tuple(alloc.tensor_shape), mybir.dt.np(alloc.dtype)
                )
            )
    n_params = len(in_names)
    n_outs = len(out_names)
    all_in_names = tuple(
        in_names + out_names + ([pname] if pname is not None else [])
    )

    def _body(*args):
        operands = list(args)
        if pname is not None:
            operands.append(bass2jax.partition_id_tensor())
        outs = bass2jax._bass_exec_p.bind(
            *operands,
            out_avals=tuple(out_avals),
            in_names=all_in_names,
            out_names=tuple(out_names),
            lowering_input_output_aliases=(),
            sim_require_finite=True,
            sim_require_nnan=True,
            nc=nc,
        )
        return tuple(outs)

    devices = jax.devices()[:N_CORES]
    mesh = Mesh(np.asarray(devices), ("core",))
    in_specs = (PartitionSpec("core"),) * (n_params + n_outs)
    out_specs = (PartitionSpec("core"),) * n_outs
    donate = tuple(range(n_params, n_params + n_outs))
    sharded = jax.jit(
        shard_map(
            _body, mesh=mesh, in_specs=in_specs, out_specs=out_specs,
            check_rep=False,
        ),
        donate_argnums=donate,
        keep_unused=True,
    )
    zshapes = [(N_CORES * av.shape[0], *av.shape[1:]) for av in out_avals]
    zdtypes = [av.dtype for av in out_avals]
    zshard = tuple([NamedSharding(mesh, PartitionSpec("core"))] * n_outs)
    make_zeros = jax.jit(
        lambda: tuple(jnp.zeros(s, d) for s, d in zip(zshapes, zdtypes)),
        out_shardings=zshard,
    )

    global _LAST_JIT
    _LAST_JIT = (sharded, make_zeros)

    from concurrent.futures import ThreadPoolExecutor

    def run(din_by_name):
        zeros = make_zeros()
        out_arrs = sharded(*[din_by_name[n] for n in in_names], *zeros)
        fetched = {}
        with ThreadPoolExecutor(N_CORES) as pool:
            for i, name in enumerate(out_names):
                shards = sorted(
                    out_arrs[i].addressable_shards,
                    key=lambda s: s.index[0].start or 0,
                )
                datas = [s.data for s in shards]
                for d in datas:
                    d.copy_to_host_async()
                # fetch + upcast per shard in threads so the f32 conversion
                # overlaps the (slow) tunnel transfer
                parts = list(
                    pool.map(
                        lambda d: np.asarray(d).astype(np.float32), datas
                    )
                )
                fetched[name] = np.concatenate(parts, axis=0)
        return fetched

    return run, in_names


def _kernel_bass(feats, rng, durations, outlen):
    import jax
    from jax.sharding import Mesh, NamedSharding, PartitionSpec

    fa, fa4, prm, jlo, jhi, L, OT, key = _prep(feats, rng, durations, outlen)

    # Start host->device transfers (async) before building/compiling the
    # kernel so they overlap.
    devices = jax.devices()[:N_CORES]
    mesh = Mesh(np.asarray(devices), ("core",))
    sh = NamedSharding(mesh, PartitionSpec("core"))
    din = {
        "fa": jax.device_put(
            fa.reshape(N_CORES * BPC, P, NCH - 1, DA), sh
        ),
        "fa4": jax.device_put(
            fa4.reshape(N_CORES * BPC, NT[NCH - 1] + 1, DA), sh
        ),
        "prm": jax.device_put(prm.reshape(N_CORES * BPC, P, NCH, 3), sh),
    }

    entry = _runners.get(key)
    if entry is None:
        nc = _build(outlen, OT, jlo, jhi, L)
        entry = _make_runner(nc)
        _runners[key] = entry
    run, _ = entry
    out = run(din)["out"]  # [B, outlen, D] f32 (upcast during fetch)
    return np.ascontiguousarray(out.reshape(B, outlen, D))


def kernel(feats, rng, durations, outlen):
    outlen = int(np.asarray(outlen))
    feats = np.asarray(feats, dtype=np.float32)
    rng = np.asarray(rng, dtype=np.float32)
    durations = np.asarray(durations)
    try:
        if _warm_thread is not None:
            _warm_thread.join()
        return _kernel_bass(feats, rng, durations, outlen)
    except Exception:
        import traceback

        traceback.print_exc()
        return _upsample_np(feats, rng, durations, outlen)


def _warmup():
    try:
        import jax

        jax.devices()
        import ml_dtypes  # noqa: F401

        from concourse import bacc, bass, bass2jax, mybir, tile

        # one-time lazy inits (cffi ISA header parse ~1.2s, engine tables)
        from concourse.isa import get_isa

        get_isa("TRN2")
        nc0 = bacc.Bacc("TRN2", target_bir_lowering=False, debug=False)
        f32 = mybir.dt.float32
        x0 = nc0.dram_tensor("x", [P, P], f32, kind="ExternalInput")
        y0 = nc0.dram_tensor("y", [P, P], f32, kind="ExternalOutput")
        with tile.TileContext(nc0) as tc0:
            with tc0.tile_pool(name="p", bufs=1) as p0:
                t0 = p0.tile([P, P], f32)
                nc0.sync.dma_start(t0[:], x0[:])
                nc0.sync.dma_start(y0[:], t0[:])
        nc0.compile()
        bass2jax.install_neuronx_cc_hook()
    except Exception:
        pass


import threading

_warm_thread = threading.Thread(target=_warmup, daemon=True)
_warm_thread.start()


# revision 39
# speedup vs baseline: 1.3011x; 1.3011x over previous
import os
import sys
import numpy as np

# GaussianUpsampler on 8 NeuronCores (Bass/Tile).
#   out[b] = rownorm(W[b]) @ feats[b],  W[b][o,t] = N(o; c[t], r[t]) + 1e-6
#   B=32, T=512, D=384, outlen ~ 2360.
# Sharding: data-parallel over batch, 4 batches per core, no collectives.
# Per core/batch: W^T band tiles (token-major = matmul lhsT layout) are
# computed on-chip (DVE z/z^2 + ACT exp) in bf16; PE contracts them against
# feats chunks augmented with a ones column (row sums for normalization) and
# a floor row (the +1e-6 * sum-over-all-tokens term, exact); DVE normalizes
# with a per-partition reciprocal. W is banded: only token chunks whose
# Gaussian centers are within +-MARGIN frames of an output tile take part
# (union across cores so one SPMD program serves all 8).

for _p in ("/opt/trn_rl_repo", "/root/.axon_site/_ro/trn_rl_repo"):
    if os.path.isdir(_p) and _p not in sys.path:
        sys.path.insert(0, _p)
        break

R2PI = float(np.sqrt(2.0 * np.pi))
B, T, D = 32, 512, 384
DA = D + 1            # feats + ones column
P = 128
N_CORES = 8
BPC = B // N_CORES    # local batches per core
NT = (127, 127, 127, 127, 4)   # tokens per K-chunk (one row left for floor)
NCH = len(NT)
TOK0 = (0, 127, 254, 381, 508)
MARGIN = 36.0

_runners = {}


def _upsample_np(feats, rng, durations, outlen):
    d = durations.astype(np.float32)
    c = d / 2.0 + np.cumsum(d, axis=-1)
    r = rng.astype(np.float32) + 1e-6
    t = np.arange(outlen, dtype=np.float32)
    z = (t[None, :, None] - c[:, None, :]) / r[:, None, :]
    w = np.exp(-0.5 * z * z) / (r[:, None, :] * R2PI) + 1e-6
    w /= w.sum(axis=2, keepdims=True)
    return np.matmul(w, feats.astype(np.float32))


def _bands(c, OT):
    """Per (slot, chunk) o-tile ranges, unioned across cores (SPMD)."""
    jlo = np.zeros((BPC, NCH), np.int64)
    jhi = np.zeros((BPC, NCH), np.int64)
    for s in range(BPC):
        for k in range(NCH):
            lo, hi = 10 ** 9, -1
            for m in range(N_CORES):
                b = m * BPC + s
                cmin = float(c[b, TOK0[k]])
                cmax = float(c[b, TOK0[k] + NT[k] - 1])
                lo = min(lo, int(np.floor((cmin - MARGIN) / P)))
                hi = max(hi, int(np.floor((cmax + MARGIN) / P)))
            jlo[s, k] = max(0, lo)
            jhi[s, k] = min(OT - 1, hi)
        # the last (tiny) chunk also carries the floor for tail frames past
        # every batch's final center
        jhi[s, NCH - 1] = OT - 1
    L = [
        [
            [k for k in range(NCH) if jlo[s, k] <= j <= jhi[s, k]]
            for j in range(OT)
        ]
        for s in range(BPC)
    ]
    for s in range(BPC):
        for j in range(OT):
            if not L[s][j]:
                raise ValueError("band coverage hole")  # -> numpy fallback
    return jlo, jhi, L


def _prep(feats, rng, durations, outlen):
    import ml_dtypes

    bf16 = ml_dtypes.bfloat16
    OT = -(-outlen // P)

    dn = durations.astype(np.float32)
    c = dn / 2.0 + np.cumsum(dn, axis=1, dtype=np.float32)
    invr = (1.0 / (rng.astype(np.float32) + 1e-6)).astype(np.float32)
    logg = np.log(invr / R2PI).astype(np.float32)
    negc = (-c * invr).astype(np.float32)

    jlo, jhi, L = _bands(c, OT)

    prm = np.zeros((N_CORES, BPC, P, NCH, 3), np.float32)
    # full chunks packed [P, NCH-1, DA] (per-partition rows contiguous =
    # 1 DMA descriptor per partition); the 5-row tail chunk rides separately
    # so 123 padding rows per batch aren't shipped through the tunnel
    fa = np.zeros((N_CORES, BPC, P, NCH - 1, DA), bf16)
    fa4 = np.zeros((N_CORES, BPC, NT[NCH - 1] + 1, DA), bf16)
    feats_bf = feats.astype(bf16)
    flo = (1e-6 * feats.astype(np.float64).sum(axis=1)).astype(np.float32)
    flo_bf = flo.astype(bf16).reshape(N_CORES, BPC, D)
    for k in range(NCH):
        n = NT[k]
        t0 = TOK0[k]
        sl = np.s_[:, t0 : t0 + n]
        prm[:, :, :n, k, 0] = invr[sl].reshape(N_CORES, BPC, n)
        prm[:, :, :n, k, 1] = negc[sl].reshape(N_CORES, BPC, n)
        prm[:, :, :n, k, 2] = logg[sl].reshape(N_CORES, BPC, n)
        if k < NCH - 1:
            fa[:, :, :n, k, :D] = feats_bf[sl].reshape(N_CORES, BPC, n, D)
            fa[:, :, :n, k, D] = bf16(1.0)
            fa[:, :, n, k, :D] = flo_bf
            fa[:, :, n, k, D] = bf16(T * 1e-6)
        else:
            fa4[:, :, :n, :D] = feats_bf[sl].reshape(N_CORES, BPC, n, D)
            fa4[:, :, :n, D] = bf16(1.0)
            fa4[:, :, n, :D] = flo_bf
            fa4[:, :, n, D] = bf16(T * 1e-6)
    key = (int(outlen), jlo.tobytes(), jhi.tobytes())
    return fa, fa4, prm, jlo, jhi, L, OT, key


def _build(outlen, OT, jlo, jhi, L):
    from concourse import bacc, bass, mybir, tile

    f32 = mybir.dt.float32
    bf16 = mybir.dt.bfloat16
    Act = mybir.ActivationFunctionType
    OPAD = OT * P

    NT4 = NT[NCH - 1] + 1
    nc = bacc.Bacc("TRN2", target_bir_lowering=False, debug=False)
    fa_d = nc.dram_tensor(
        "fa", [BPC, P, NCH - 1, DA], bf16, kind="ExternalInput"
    )
    fa4_d = nc.dram_tensor("fa4", [BPC, NT4, DA], bf16, kind="ExternalInput")
    prm_d = nc.dram_tensor("prm", [BPC, P, NCH, 3], f32, kind="ExternalInput")
    # bf16 output halves the device->host transfer (the wall-clock
    # bottleneck over the axon tunnel); host upcasts to f32.
    out_d = nc.dram_tensor("out", [BPC, outlen, D], bf16, kind="ExternalOutput")
    JF = outlen // P  # full o-tiles; tile JF (if any) is partial

    with tile.TileContext(nc) as tc:
        with (
            tc.tile_pool(name="const", bufs=1) as cpool,
            tc.tile_pool(name="wz", bufs=3) as wzpool,
            tc.tile_pool(name="fap", bufs=2) as fapool,
            tc.tile_pool(name="op", bufs=2) as opool,
            tc.tile_pool(name="rc", bufs=8) as rcpool,
            tc.tile_pool(name="ps", bufs=8, space=bass.MemorySpace.PSUM) as pspool,
        ):
            # dependency-free ACT op up front so the ~2.7us table load runs
            # at t=0, overlapped with iota/param DMAs instead of serialized
            # before the first real Square
            dum_t = cpool.tile([1, 2], f32)
            nc.vector.memset(dum_t[:], 0.0)
            nc.scalar.activation(dum_t[:], dum_t[:], Act.Exp)
            # f32 iota is exact up to 2^24; avoids an int32 staging copy
            iota_t = cpool.tile([P, OPAD], f32)
            nc.gpsimd.iota(
                iota_t[:],
                pattern=[[1, OPAD]],
                base=0,
                channel_multiplier=0,
                allow_small_or_imprecise_dtypes=True,
            )
            for s in range(BPC):
                prm_t = fapool.tile([P, NCH, 3], f32, tag="prm")
                nc.sync.dma_start(prm_t[:], prm_d[s])
                fa_t = fapool.tile([P, NCH - 1, DA], bf16, tag="fa")
                nc.sync.dma_start(fa_t[:], fa_d[s])
                fa4_t = fapool.tile([NT4, DA], bf16, tag="fa4")
                nc.sync.dma_start(fa4_t[:], fa4_d[s])
                w_ts = []
                for k in range(NCH):
                    wk = (int(jhi[s, k]) - int(jlo[s, k]) + 1) * P
                    o0 = int(jlo[s, k]) * P
                    # z^2 = (o*invr - c*invr)^2 in one ACT pass (per-partition
                    # scale/bias); Square+Exp share one table set.
                    z_t = wzpool.tile([P, wk], f32, tag=f"z{k}")
                    nc.scalar.activation(
                        z_t[:],
                        iota_t[:, o0 : o0 + wk],
                        Act.Square,
                        bias=prm_t[:, k, 1:2],
                        scale=prm_t[:, k, 0:1],
                    )
                    w_t = wzpool.tile([P, wk], bf16, tag=f"w{k}")
                    # floor row NT[k]: prm rows are zeroed there, so this
                    # same pass writes exp(-0.5*0 + 0) = 1.0 — the ones row
                    # the floor matmul term needs.
                    nc.scalar.activation(
                        w_t[:], z_t[:], Act.Exp, bias=prm_t[:, k, 2:3], scale=-0.5
                    )
                    w_ts.append(w_t)
                obig_t = opool.tile([P, OT, D], bf16, tag="obig")
                for j in range(OT):
                    ks = L[s][j]
                    ps_t = pspool.tile([P, DA], f32)
                    for i, k in enumerate(ks):
                        rows = NT[k] + (1 if i == 0 else 0)
                        col0 = (j - int(jlo[s, k])) * P
                        rhs = (
                            fa_t[0:rows, k, :]
                            if k < NCH - 1
                            else fa4_t[0:rows, :]
                        )
                        nc.tensor.matmul(
                            ps_t[:],
                            w_ts[k][0:rows, col0 : col0 + P],
                            rhs,
                            start=(i == 0),
                            stop=(i == len(ks) - 1),
                        )
                    rec_t = rcpool.tile([P, 1], f32, tag="rec")
                    nc.vector.reciprocal(rec_t[:], ps_t[:, D:DA])
                    # PSUM drain + normalize + bf16 convert in one pass;
                    # alternate DVE/ACT to balance engine load
                    if j % 13 == 12:
                        nc.scalar.activation(
                            obig_t[:, j, :],
                            ps_t[:, 0:D],
                            Act.Copy,
                            scale=rec_t[:],
                        )
                    else:
                        nc.vector.tensor_scalar_mul(
                            obig_t[:, j, :], ps_t[:, 0:D], rec_t[:]
                        )
                # two DMAs per slot instead of one per o-tile: HWDGE fixed
                # overhead (~625ns/instr) dominated the DMA path otherwise
                nc.sync.dma_start(
                    out_d[s, 0 : JF * P, :].rearrange(
                        "(j p) d -> p j d", p=P
                    ),
                    obig_t[:, 0:JF, :],
                )
                if outlen > JF * P:
                    nc.sync.dma_start(
                        out_d[s, JF * P : outlen, :],
                        obig_t[0 : outlen - JF * P, JF, :],
                    )
    nc.compile()
    return nc


def _make_runner(nc):
    """One-time jit of the SPMD bass_exec call; later calls only transfer."""
    import jax
    import jax.numpy as jnp
    from jax.experimental.shard_map import shard_map
    from jax.sharding import Mesh, NamedSharding, PartitionSpec

    from concourse import bass2jax, mybir

    bass2jax.install_neuronx_cc_hook()
    assert nc.dbg_addr is None
    pname = nc.partition_id_tensor.name if nc.partition_id_tensor else None

    in_names, out_names, out_avals = [], [], []
    for alloc in nc.m.functions[0].allocations:
        if not isinstance(alloc, mybir.MemoryLocationSet):
            continue
        name = alloc.memorylocations[0].name
        if alloc.kind == "ExternalInput":
            if name != pname:
                in_names.append(name)
        elif alloc.kind == "ExternalOutput":
            out_names.append(name)
            out_avals.append(
                jax.core.ShapedArray(
                    tuple(alloc.tensor_shape), mybir.dt.np(alloc.dtype)
                )
            )
    n_params = len(in_names)
    n_outs = len(out_names)
    all_in_names = tuple(
        in_names + out_names + ([pname] if pname is not None else [])
    )

    def _body(*args):
        operands = list(args)
        if pname is not None:
            operands.append(bass2jax.partition_id_tensor())
        outs = bass2jax._bass_exec_p.bind(
            *operands,
            out_avals=tuple(out_avals),
            in_names=all_in_names,
            out_names=tuple(out_names),
            lowering_input_output_aliases=(),
            sim_require_finite=True,
            sim_require_nnan=True,
            nc=nc,
        )
        return tuple(outs)

    devices = jax.devices()[:N_CORES]
    mesh = Mesh(np.asarray(devices), ("core",))
    in_specs = (PartitionSpec("core"),) * (n_params + n_outs)
    out_specs = (PartitionSpec("core"),) * n_outs
    donate = tuple(range(n_params, n_params + n_outs))
    sharded = jax.jit(
        shard_map(
            _body, mesh=mesh, in_specs=in_specs, out_specs=out_specs,
            check_rep=False,
        ),
        donate_argnums=donate,
        keep_unused=True,
    )
    zshapes = [(N_CORES * av.shape[0], *av.shape[1:]) for av in out_avals]
    zdtypes = [av.dtype for av in out_avals]
    zshard = tuple([NamedSharding(mesh, PartitionSpec("core"))] * n_outs)
    make_zeros = jax.jit(
        lambda: tuple(jnp.zeros(s, d) for s, d in zip(zshapes, zdtypes)),
        out_shardings=zshard,
    )

    global _LAST_JIT
    _LAST_JIT = (sharded, make_zeros)

    from concurrent.futures import ThreadPoolExecutor

    def run(din_by_name):
        zeros = make_zeros()
        out_arrs = sharded(*[din_by_name[n] for n in in_names], *zeros)
        fetched = {}
        with ThreadPoolExecutor(N_CORES) as pool:
            for i, name in enumerate(out_names):
                shards = sorted(
                    out_arrs[i].addressable_shards,
                    key=lambda s: s.index[0].start or 0,
                )
                datas = [s.data for s in shards]
                for d in datas:
                    d.copy_to_host_async()
                # fetch + upcast per shard in threads so the f32 conversion
                # overlaps the (slow) tunnel transfer
                parts = list(
                    pool.map(
                        lambda d: np.asarray(d).astype(np.float32), datas
                    )
                )
                fetched[name] = np.concatenate(parts, axis=0)
        return fetched

    return run, in_names


def _kernel_bass(feats, rng, durations, outlen):
    import jax
    from jax.sharding import Mesh, NamedSharding, PartitionSpec

    fa, fa4, prm, jlo, jhi, L, OT, key = _prep(feats, rng, durations, outlen)

    # Start host->device transfers (async) before building/compiling the
    # kernel so they overlap.
    devices = jax.devices()[:N_CORES]
    mesh = Mesh(np.asarray(devices), ("core",))
    sh = NamedSharding(mesh, PartitionSpec("core"))
    din = {
        "fa": jax.device_put(
            fa.reshape(N_CORES * BPC, P, NCH - 1, DA), sh
        ),
        "fa4": jax.device_put(
            fa4.reshape(N_CORES * BPC, NT[NCH - 1] + 1, DA), sh
        ),
        "prm": jax.device_put(prm.reshape(N_CORES * BPC, P, NCH, 3), sh),
    }

    entry = _runners.get(key)
    if entry is None:
        nc = _build(outlen, OT, jlo, jhi, L)
        entry = _make_runner(nc)
        _runners[key] = entry
    run, _ = entry
    out = run(din)["out"]  # [B, outlen, D] f32 (upcast during fetch)
    return np.ascontiguousarray(out.reshape(B, outlen, D))


def kernel(feats, rng, durations, outlen):
    outlen = int(np.asarray(outlen))
    feats = np.asarray(feats, dtype=np.float32)
    rng = np.asarray(rng, dtype=np.float32)
    durations = np.asarray(durations)
    try:
        if _warm_thread is not None:
            _warm_thread.join()
        return _kernel_bass(feats, rng, durations, outlen)
    except Exception:
        import traceback

        traceback.print_exc()
        return _upsample_np(feats, rng, durations, outlen)


def _warmup():
    try:
        import jax

        try:
            jax.config.update(
                "jax_compilation_cache_dir", "/root/.cache/jaxcomp"
            )
            jax.config.update(
                "jax_persistent_cache_min_compile_time_secs", 0.0
            )
        except Exception:
            pass
        jax.devices()
        import ml_dtypes  # noqa: F401

        from concourse import bacc, bass, bass2jax, mybir, tile

        # one-time lazy inits (cffi ISA header parse ~1.2s, engine tables)
        from concourse.isa import get_isa

        get_isa("TRN2")
        nc0 = bacc.Bacc("TRN2", target_bir_lowering=False, debug=False)
        f32 = mybir.dt.float32
        x0 = nc0.dram_tensor("x", [P, P], f32, kind="ExternalInput")
        y0 = nc0.dram_tensor("y", [P, P], f32, kind="ExternalOutput")
        with tile.TileContext(nc0) as tc0:
            with tc0.tile_pool(name="p", bufs=1) as p0:
                t0 = p0.tile([P, P], f32)
                nc0.sync.dma_start(t0[:], x0[:])
                nc0.sync.dma_start(y0[:], t0[:])
        nc0.compile()
        bass2jax.install_neuronx_cc_hook()
    except Exception:
        pass


import threading

_warm_thread = threading.Thread(target=_warmup, daemon=True)
_warm_thread.start()
